# revision 28
# baseline (speedup 1.0000x reference)
"""Trainium2 Bass kernel for PointTransformerNorm (B=8, N=2048).

Data-parallel: one sample per NeuronCore across 8 cores; all params
replicated.  Per-core program (channel-on-partition layouts):

  x0[3,N] -> conv1+bn+relu -> conv2+bn+relu -> 4x offset-attention ->
  (feats spilled to DRAM) -> convf+bn+lrelu -> global max/avg ->
  rank-1 gf trick -> ws1+bn+relu -> ws2+bn+relu -> ws3+bias ->
  L2-normalize -> out[N,3]

Attention per layer (a = wqk@x [64,N], e = exp(a_i^T a) row blocks):
  the row softmax and the extra column renorm fold into matmul
  operands: u = 1/rowsum(e) scales vT (rhs), colsum rides along as an
  extra rhs column, so attention probabilities are never materialized.
  x_r is produced transposed ([n,256] blocks) and PE-transposed back.

FAST=True runs matmuls in float32r (fp32 with 11-bit mantissa, 4x the
PE throughput of fp32).  Every f32r matmul operand must be *produced*
with an f32r-typed output (engines round on store); host-side weights
are pre-rounded with the bit-exact RNE transform.  The normalize tail
stays plain fp32.
"""

import sys

sys.path.insert(0, "/opt/trn_rl_repo")

import numpy as np
from contextlib import ExitStack

import concourse.bacc as bacc
import concourse.tile as tile
import concourse.mybir as mybir

F32 = mybir.dt.float32
F32R = mybir.dt.float32r
AF = mybir.ActivationFunctionType
ALU = mybir.AluOpType
AX = mybir.AxisListType

N = 2048
NT = 16          # 128-row blocks over N
CH = 512         # psum chunk (fp32 moving max)
MC = N // CH     # 4
EPS = 1e-5
E_BUFS = 8       # live e-blocks (8KB/partition each)
NCORES = 8
FAST = True      # float32r matmuls (4x PE) vs exact fp32
VW = 258         # vtu width: 256 channels + colsum col + pad (even for f32r)

DT_W = F32R if FAST else F32


def r32(ap):
    """View a matmul operand as f32r (values must be on the f32r grid)."""
    return ap.bitcast(F32R) if FAST else ap


def rw(ap):
    """Producer-side: write rounded to the f32r grid."""
    return ap.bitcast(F32R) if FAST else ap


def round_f32r(a):
    """Bit-exact host equivalent of walrus fp32_to_fp32r (RNE to 11-bit
    mantissa); keeps inf/nan."""
    a = np.ascontiguousarray(a, np.float32)
    if not FAST:
        return a
    u = a.reshape(-1).view(np.uint32)
    r = (u + (0x7FF + ((u >> 12) & 1))) & 0xFFFFF000
    keep = (u & 0x7F800000) == 0x7F800000
    return (np.where(keep, u, r).astype(np.uint32)
            .view(np.float32).reshape(a.shape))


# ---------------------------------------------------------------- host prep

def _cols(v):
    """[O] channel vector -> [128, O//128] (partition, out-tile) layout."""
    v = np.asarray(v, np.float32)
    ot = v.size // 128
    return np.ascontiguousarray(v.reshape(ot, 128).T)


def _bnfold(bn, extra=None):
    g = np.asarray(bn["g"], np.float32)
    b = np.asarray(bn["b"], np.float32)
    m = np.asarray(bn["m"], np.float32)
    v = np.asarray(bn["v"], np.float32)
    s = g / np.sqrt(v + EPS)
    c = b - s * m
    if extra is not None:
        c = c + s * np.asarray(extra, np.float32)
    return s, c


# weights fed to f32r matmuls (pre-rounded on the host, DRAM dtype f32r)
F32R_NAMES = set()


def prep_weights(params):
    H = {}

    def T(w):
        return np.ascontiguousarray(np.asarray(w, np.float32).T)

    H["w1T"] = T(params["w1"])                   # [3,256]
    s, c = _bnfold(params["bn1"])
    H["bn1s"], H["bn1c"] = _cols(s), _cols(c)
    H["w2T"] = T(params["w2"])                   # [256,256]
    s, c = _bnfold(params["bn2"])
    H["bn2s"], H["bn2c"] = _cols(s), _cols(c)
    for l, sa in enumerate(params["sa"]):
        H[f"wqkT{l}"] = T(sa["wqk"])             # [256,64]
        H[f"wvT{l}"] = T(sa["wv"])               # [256,256]
        H[f"bvrow{l}"] = np.ascontiguousarray(
            np.asarray(sa["bv"], np.float32)[None, :])
        H[f"wtT{l}"] = T(sa["wt"])               # [256,256]
        s, c = _bnfold(sa["bn"], extra=sa["bt"])
        H[f"bnts{l}"], H[f"bntc{l}"] = _cols(s), _cols(c)
    H["wfT"] = T(params["wf"])                   # [1024,1024]
    s, c = _bnfold(params["bnf"])
    H["bnfs"], H["bnfc"] = _cols(s), _cols(c)
    ws1 = np.asarray(params["ws1"], np.float32)
    H["ws1bT"] = np.ascontiguousarray(ws1[:, 1024:].T)          # [1024,512]
    H["s1row"] = np.ascontiguousarray(ws1[:, :1024].sum(axis=1)[None, :])
    s, c = _bnfold(params["bns1"], extra=params["bs1"])
    H["bns1s"], H["bns1c"] = _cols(s), _cols(c)
    H["ws2T"] = T(params["ws2"])                 # [512,256]
    s, c = _bnfold(params["bns2"], extra=params["bs2"])
    H["bns2s"], H["bns2c"] = _cols(s), _cols(c)
    H["ws3T"] = T(params["ws3"])                 # [256,3]
    H["bs3row"] = np.ascontiguousarray(
        np.asarray(params["bs3"], np.float32)[None, :])
    H["ident"] = np.eye(128, dtype=np.float32)
    H["ones1"] = np.ones((1, 128), np.float32)
    H["ones3"] = np.ones((3, 1), np.float32)
    H["ones1f"] = np.ones((1, 4), np.float32)
    H["onesrow"] = np.ones((1, CH), np.float32)
    out = {}
    for k, v in H.items():
        v = np.ascontiguousarray(np.asarray(v, np.float32))
        if k in F32R_NAMES:
            v = round_f32r(v)
        out[k] = v
    return out


WEIGHT_SHAPES = {
    "w1T": (3, 256), "bn1s": (128, 2), "bn1c": (128, 2),
    "w2T": (256, 256), "bn2s": (128, 2), "bn2c": (128, 2),
    "wfT": (1024, 1024), "bnfs": (128, 8), "bnfc": (128, 8),
    "ws1bT": (1024, 512), "s1row": (1, 512),
    "bns1s": (128, 4), "bns1c": (128, 4),
    "ws2T": (512, 256), "bns2s": (128, 2), "bns2c": (128, 2),
    "ws3T": (256, 3), "bs3row": (1, 3),
    "ident": (128, 128), "ones1": (1, 128), "ones3": (3, 1),
    "ones1f": (1, 4), "onesrow": (1, CH),
}
for _l in range(4):
    WEIGHT_SHAPES.update({
        f"wqkT{_l}": (256, 64), f"wvT{_l}": (256, 256),
        f"bvrow{_l}": (1, 256), f"wtT{_l}": (256, 256),
        f"bnts{_l}": (128, 2), f"bntc{_l}": (128, 2),
    })

F32R_NAMES.update({"w1T", "w2T", "wfT", "ws1bT", "s1row", "ws2T", "ws3T",
                   "bs3row", "ones1", "onesrow"})
for _l in range(4):
    F32R_NAMES.update({f"wqkT{_l}", f"wvT{_l}", f"bvrow{_l}", f"wtT{_l}"})


def dram_dtype(name):
    return DT_W if name in F32R_NAMES else F32


# ---------------------------------------------------------------- program

def _load_kt(nc, pool, dr, name, kt, osz, dtype=F32):
    """Load [kt*128, osz] DRAM weight as SBUF [128, kt, osz]."""
    t = pool.tile([128, kt, osz], dtype, name=name)
    nc.sync.dma_start(out=t, in_=dr[name][:].rearrange(
        "(kt p) o -> p kt o", p=128))
    return t


def _sa_layer(nc, tc, l, x_t, xpool, dpool, dr, ident, ones1):
    """One offset-attention layer; returns the new x tile [128, 2, N]."""
    phases = []
    j0 = 0
    while j0 < NT:
        phases.append((j0, min(j0 + E_BUFS, NT)))
        j0 += E_BUFS
    n_ph = len(phases)

    with ExitStack() as octx:
        lw = octx.enter_context(tc.tile_pool(name=f"lw{l}", bufs=1))
        sp = octx.enter_context(tc.tile_pool(name=f"sp{l}", bufs=1))
        accp = octx.enter_context(tc.tile_pool(name=f"ac{l}", bufs=16))
        tiny = octx.enter_context(tc.tile_pool(name=f"tn{l}", bufs=4))

        wqkT = _load_kt(nc, lw, dr, f"wqkT{l}", 2, 64, DT_W)
        wvT = _load_kt(nc, lw, dr, f"wvT{l}", 2, 256, DT_W)
        wtT = _load_kt(nc, lw, dr, f"wtT{l}", 2, 256, DT_W)
        bvrow = lw.tile([1, 256], DT_W, name=f"bvrow{l}")
        nc.sync.dma_start(out=bvrow, in_=dr[f"bvrow{l}"][:])
        bnts = lw.tile([128, 2], F32, name=f"bnts{l}")
        nc.sync.dma_start(out=bnts, in_=dr[f"bnts{l}"][:])
        bntc = lw.tile([128, 2], F32, name=f"bntc{l}")
        nc.sync.dma_start(out=bntc, in_=dr[f"bntc{l}"][:])

        # ---- a = wqk @ x : [64, N]
        a_sb = sp.tile([64, N], F32, name=f"a{l}")
        with tc.tile_pool(name=f"aps{l}", bufs=1, space="PSUM") as aps:
            apsum = aps.tile([64, N], F32, name=f"aps{l}")
            for mc in range(MC):
                sl = slice(mc * CH, (mc + 1) * CH)
                for k in range(2):
                    nc.tensor.matmul(apsum[:, sl], lhsT=r32(wqkT[:, k, :]),
                                     rhs=r32(x_t[:, k, sl]),
                                     start=(k == 0), stop=(k == 1))
            nc.vector.tensor_copy(rw(a_sb), apsum)

        e_tiles = [None] * NT
        vtu_tiles = [None] * NT
        acc_tiles = [None] * NT

        with ExitStack() as ictx:
            eps_ = ictx.enter_context(
                tc.tile_pool(name=f"eps{l}", bufs=2, space="PSUM"))
            vps = ictx.enter_context(
                tc.tile_pool(name=f"vps{l}", bufs=2, space="PSUM"))
            xps = ictx.enter_context(
                tc.tile_pool(name=f"xps{l}", bufs=2, space="PSUM"))
            ep = ictx.enter_context(tc.tile_pool(name=f"ep{l}", bufs=E_BUFS))
            vt = ictx.enter_context(tc.tile_pool(name=f"vt{l}",
                                                 bufs=E_BUFS))
            ut = ictx.enter_context(tc.tile_pool(name=f"ut{l}",
                                                 bufs=E_BUFS + 2))

            def emit_j(j):
                # energy row-block j + exp + rowsum
                e_t = ep.tile([128, N], F32, tag="e", name=f"e{l}_{j}")
                rs = tiny.tile([128, 2], F32, tag="rs", name=f"rs{l}_{j}")
                for h in range(2):
                    pse = eps_.tile([128, 1024], F32, tag="e_ps",
                                    name=f"eps{l}_{j}_{h}")
                    for q in range(2):
                        col = (2 * h + q) * CH
                        nc.tensor.matmul(
                            pse[:, q * CH:(q + 1) * CH],
                            lhsT=r32(a_sb[:, j * 128:(j + 1) * 128]),
                            rhs=r32(a_sb[:, col:col + CH]),
                            start=True, stop=True)
                    nc.scalar.activation(rw(e_t[:, h * 1024:(h + 1) * 1024]),
                                         pse, AF.Exp,
                                         accum_out=rs[:, h:h + 1])
                u_j = ut.tile([128, 1], F32, tag="u", name=f"u{l}_{j}")
                rsum = tiny.tile([128, 1], F32, tag="rsum",
                                 name=f"rsum{l}_{j}")
                nc.vector.tensor_add(rsum, rs[:, 0:1], rs[:, 1:2])
                nc.vector.reciprocal(u_j, rsum)
                # vT block j, scaled by u, with u as the colsum column
                psv = vps.tile([128, 256], F32, tag="v_ps",
                               name=f"vps{l}_{j}")
                for k in range(2):
                    nc.tensor.matmul(
                        psv, lhsT=r32(x_t[:, k, j * 128:(j + 1) * 128]),
                        rhs=r32(wvT[:, k, :]), start=(k == 0), stop=False)
                nc.tensor.matmul(psv, lhsT=r32(ones1), rhs=r32(bvrow),
                                 start=False, stop=True)
                vtu_j = vt.tile([128, VW], F32, tag="vtu",
                                name=f"vtu{l}_{j}")
                nc.vector.tensor_scalar_mul(rw(vtu_j[:, 0:256]), psv, u_j)
                for cx in range(256, VW):
                    nc.vector.tensor_copy(rw(vtu_j[:, cx:cx + 1]), u_j)
                e_tiles[j], vtu_tiles[j] = e_t, vtu_j

            for pi, (p0, p1) in enumerate(phases):
                for j in range(p0, p1):
                    emit_j(j)
                last = pi == n_ph - 1
                for i in range(NT):
                    psx = xps.tile([128, VW], F32, tag="xr_ps",
                                   name=f"xps{l}_{pi}_{i}")
                    for j in range(p0, p1):
                        nc.tensor.matmul(
                            psx,
                            lhsT=r32(e_tiles[j][:, i * 128:(i + 1) * 128]),
                            rhs=r32(vtu_tiles[j]),
                            start=(j == p0), stop=(j == p1 - 1))
                    if pi == 0 and not last:
                        acc_tiles[i] = accp.tile([128, VW], F32, tag="acc",
                                                 name=f"acc{l}_{i}")
                        nc.vector.tensor_copy(acc_tiles[i], psx)
                    elif not last:
                        nc.vector.tensor_add(acc_tiles[i], psx, acc_tiles[i])
                    else:
                        cs = tiny.tile([128, 1], F32, tag="cs",
                                       name=f"cs{l}_{i}")
                        if n_ph > 1:
                            nc.vector.scalar_tensor_tensor(
                                out=cs, in0=psx[:, 256:257], scalar=1e-9,
                                in1=acc_tiles[i][:, 256:257],
                                op0=ALU.add, op1=ALU.add)
                            nc.vector.tensor_add(psx[:, 0:256], psx[:, 0:256],
                                                 acc_tiles[i][:, 0:256])
                        else:
                            acc_tiles[i] = accp.tile([128, VW], F32,
                                                     tag="acc",
                                                     name=f"acc{l}_{i}")
                            nc.vector.tensor_scalar_add(cs, psx[:, 256:257],
                                                        1e-9)
                        u2 = tiny.tile([128, 1], F32, tag="u2",
                                       name=f"u2{l}_{i}")
                        nc.vector.reciprocal(u2, cs)
                        # final x_r^T lands in the acc tile (cols 0..255)
                        nc.vector.tensor_scalar_mul(
                            acc_tiles[i][:, 0:256], psx[:, 0:256], u2)

        # ---- transpose x_r back, d = x - x_r, t-conv, bn+relu, residual
        with tc.tile_pool(name=f"trp{l}", bufs=2, space="PSUM") as trps, \
             tc.tile_pool(name=f"tps{l}", bufs=2, space="PSUM") as tps:
            d_t = dpool.tile([128, 2, N], F32, tag="d", name=f"d{l}")
            for i in range(NT):
                for cc in range(2):
                    pst = trps.tile([128, 128], F32, tag="tr_ps",
                                    name=f"tr{l}_{i}_{cc}")
                    nc.tensor.transpose(
                        pst, acc_tiles[i][:, cc * 128:(cc + 1) * 128], ident)
                    sl = slice(i * 128, (i + 1) * 128)
                    nc.vector.scalar_tensor_tensor(
                        out=rw(d_t[:, cc, sl]), in0=pst, scalar=-1.0,
                        in1=x_t[:, cc, sl], op0=ALU.mult, op1=ALU.add)
            xn = xpool.tile([128, 2, N], F32, tag="xresid", bufs=2,
                            name=f"feat{l}")
            for o in range(2):
                for mc in range(MC):
                    sl = slice(mc * CH, (mc + 1) * CH)
                    ps = tps.tile([128, CH], F32, tag="t_ps",
                                  name=f"tps{l}_{o}_{mc}")
                    for k in range(2):
                        nc.tensor.matmul(
                            ps, lhsT=r32(wtT[:, k, o * 128:(o + 1) * 128]),
                            rhs=r32(d_t[:, k, sl]),
                            start=(k == 0), stop=(k == 1))
                    tmp = sp.tile([128, CH], F32, tag="ttmp", bufs=2,
                                  name=f"ttmp{l}_{o}_{mc}")
                    nc.scalar.activation(tmp, ps, AF.Relu,
                                         bias=bntc[:, o:o + 1],
                                         scale=bnts[:, o:o + 1])
                    nc.vector.tensor_add(rw(xn[:, o, sl]), x_t[:, o, sl],
                                         tmp)
    return xn


def build_program(nc, tc, dr):
    with ExitStack() as ctx:
        wp = ctx.enter_context(tc.tile_pool(name="wp", bufs=1))
        ident = wp.tile([128, 128], F32, name="ident")
        nc.sync.dma_start(out=ident, in_=dr["ident"][:])
        ones1 = wp.tile([1, 128], DT_W, name="ones1")
        nc.sync.dma_start(out=ones1, in_=dr["ones1"][:])
        ones3 = wp.tile([3, 1], F32, name="ones3")
        nc.sync.dma_start(out=ones3, in_=dr["ones3"][:])
        ones1f = wp.tile([1, 4], F32, name="ones1f")
        nc.sync.dma_start(out=ones1f, in_=dr["ones1f"][:])
        onesrow = wp.tile([1, CH], DT_W, name="onesrow")
        nc.sync.dma_start(out=onesrow, in_=dr["onesrow"][:])

        # x-residual rotation (x2 + feat0..3 share 2 slots) and the
        # DRAM spill tensors for the tail concat
        xpool = ctx.enter_context(tc.tile_pool(name="xpool", bufs=2))
        fdram_p = ctx.enter_context(
            tc.tile_pool(name="fdram", bufs=1, space="DRAM"))
        fdram = [fdram_p.tile([128, 2, N], DT_W, name=f"fd{l}")
                 for l in range(4)]

        # ---------------- head: conv1 + conv2
        with tc.tile_pool(name="hw", bufs=1) as hw, \
             tc.tile_pool(name="hps", bufs=2, space="PSUM") as hps:
            w1T = hw.tile([3, 256], DT_W, name="w1T")
            nc.sync.dma_start(out=w1T, in_=dr["w1T"][:])
            w2T = _load_kt(nc, hw, dr, "w2T", 2, 256, DT_W)
            bns = {}
            for nm in ("bn1s", "bn1c", "bn2s", "bn2c"):
                bns[nm] = hw.tile([128, 2], F32, name=nm)
                nc.sync.dma_start(out=bns[nm], in_=dr[nm][:])
            x0 = hw.tile([3, N], DT_W, name="x0")
            nc.sync.dma_start(out=x0, in_=dr["xin"][:].rearrange("n c -> c n"))
            x1 = hw.tile([128, 2, N], F32, name="x1")
            for o in range(2):
                for mc in range(MC):
                    sl = slice(mc * CH, (mc + 1) * CH)
                    ps = hps.tile([128, CH], F32, tag="h_ps",
                                  name=f"h1_{o}_{mc}")
                    nc.tensor.matmul(ps,
                                     lhsT=r32(w1T[:, o * 128:(o + 1) * 128]),
                                     rhs=r32(x0[:, sl]),
                                     start=True, stop=True)
                    nc.scalar.activation(rw(x1[:, o, sl]), ps, AF.Relu,
                                         bias=bns["bn1c"][:, o:o + 1],
                                         scale=bns["bn1s"][:, o:o + 1])
            x2 = xpool.tile([128, 2, N], F32, tag="xresid", bufs=2,
                            name="x2")
            for o in range(2):
                for mc in range(MC):
                    sl = slice(mc * CH, (mc + 1) * CH)
                    ps = hps.tile([128, CH], F32, tag="h_ps",
                                  name=f"h2_{o}_{mc}")
                    for k in range(2):
                        nc.tensor.matmul(
                            ps, lhsT=r32(w2T[:, k, o * 128:(o + 1) * 128]),
                            rhs=r32(x1[:, k, sl]),
                            start=(k == 0), stop=(k == 1))
                    nc.scalar.activation(rw(x2[:, o, sl]), ps, AF.Relu,
                                         bias=bns["bn2c"][:, o:o + 1],
                                         scale=bns["bn2s"][:, o:o + 1])

        # ---------------- 4 offset-attention layers
        x_t = x2
        with ExitStack() as dctx:
            dpool = dctx.enter_context(tc.tile_pool(name="dp", bufs=1))
            for l in range(4):
                x_t = _sa_layer(nc, tc, l, x_t, xpool, dpool, dr,
                                ident, ones1)
                # spill the new feature block for the tail concat
                nc.sync.dma_start(out=fdram[l][:], in_=r32(x_t))

        # ---------------- tail
        with tc.tile_pool(name="tw", bufs=1) as tw, \
             tc.tile_pool(name="ts", bufs=1) as ts, \
             tc.tile_pool(name="fck", bufs=2) as fck, \
             tc.tile_pool(name="st", bufs=2) as st:
            wfk = [None] * 8
            ws1k = [None] * 8
            for kt in range(8):
                wfk[kt] = tw.tile([128, 1024], DT_W, name=f"wfT{kt}")
                nc.sync.dma_start(
                    out=wfk[kt],
                    in_=dr["wfT"][:].rearrange("(kt p) o -> kt p o",
                                               p=128)[kt])
            bnfs = tw.tile([128, 8], F32, name="bnfs")
            nc.sync.dma_start(out=bnfs, in_=dr["bnfs"][:])
            bnfc = tw.tile([128, 8], F32, name="bnfc")
            nc.sync.dma_start(out=bnfc, in_=dr["bnfc"][:])
            for kt in range(8):
                ws1k[kt] = tw.tile([128, 512], DT_W, name=f"ws1bT{kt}")
                nc.sync.dma_start(
                    out=ws1k[kt],
                    in_=dr["ws1bT"][:].rearrange("(kt p) o -> kt p o",
                                                 p=128)[kt])
            s1row = tw.tile([1, 512], DT_W, name="s1row")
            nc.sync.dma_start(out=s1row, in_=dr["s1row"][:])
            small = {}
            for nm, shp, dt_ in (("bns1s", (128, 4), F32),
                                 ("bns1c", (128, 4), F32),
                                 ("bns2s", (128, 2), F32),
                                 ("bns2c", (128, 2), F32),
                                 ("bs3row", (1, 3), DT_W)):
                small[nm] = tw.tile(list(shp), dt_, name=nm)
                nc.sync.dma_start(out=small[nm], in_=dr[nm][:])
            ws2T = _load_kt(nc, tw, dr, "ws2T", 4, 256, DT_W)
            ws3T = _load_kt(nc, tw, dr, "ws3T", 2, 3, DT_W)

            # convf + bn + leaky-relu (+ per-chunk row sums for the mean);
            # feats stream back from DRAM per 512-chunk
            xf = ts.tile([128, 8, N], F32, name="xf")
            sums = ts.tile([128, 8, MC], F32, name="sums")
            with tc.tile_pool(name="fps", bufs=4, space="PSUM") as fps:
                for mc in range(MC):
                    sl = slice(mc * CH, (mc + 1) * CH)
                    fcs = []
                    for hh in range(2):
                        fc = fck.tile([128, 4, CH], DT_W, tag="fc",
                                      name=f"fc{mc}_{hh}")
                        for kq in range(4):
                            kt = hh * 4 + kq
                            nc.sync.dma_start(
                                out=fc[:, kq, :],
                                in_=fdram[kt // 2][:, kt % 2, sl])
                        fcs.append(fc)
                    for ot in range(8):
                        ps = fps.tile([128, CH], F32, tag="f_ps",
                                      name=f"f_{ot}_{mc}")
                        for kt in range(8):
                            nc.tensor.matmul(
                                ps,
                                lhsT=r32(wfk[kt][:, ot * 128:(ot + 1) * 128]),
                                rhs=r32(fcs[kt // 4][:, kt % 4, :]),
                                start=(kt == 0), stop=(kt == 7))
                        t1 = fck.tile([128, CH], F32, tag="t1", bufs=2,
                                      name=f"t1_{ot}_{mc}")
                        nc.scalar.activation(t1, ps, AF.Identity,
                                             bias=bnfc[:, ot:ot + 1],
                                             scale=bnfs[:, ot:ot + 1])
                        # leaky relu: max(0.2*y, y), with row-sum accum
                        nc.vector.scalar_tensor_tensor(
                            out=rw(xf[:, ot, sl]), in0=t1, scalar=0.2,
                            in1=t1, op0=ALU.mult, op1=ALU.max,
                            accum_out=sums[:, ot, mc:mc + 1])

            # global features -> one [1, 2048] row (max | mean)
            xmax = ts.tile([128, 8], F32, name="xmax")
            xsum = ts.tile([128, 8], F32, name="xsum")
            for ot in range(8):
                nc.vector.reduce_max(rw(xmax[:, ot:ot + 1]), xf[:, ot, :],
                                     axis=AX.X)
                nc.vector.reduce_sum(xsum[:, ot:ot + 1], sums[:, ot, :],
                                     axis=AX.X)
            nc.vector.tensor_scalar_mul(rw(xsum), xsum, 1.0 / N)
            gf_row = st.tile([1, N], DT_W, name="gf_row", bufs=1)
            with tc.tile_pool(name="dgp", bufs=1, space="DRAM") as dgp:
                dxm = dgp.tile([128, 8], DT_W, name="dxm")
                dxs = dgp.tile([128, 8], DT_W, name="dxs")
                nc.sync.dma_start(out=dxm[:], in_=r32(xmax))
                nc.sync.dma_start(out=dxs[:], in_=r32(xsum))
                nc.sync.dma_start(
                    out=gf_row[0:1, 0:1024].rearrange("a (o p) -> a o p",
                                                      p=128),
                    in_=dxm[:].rearrange("p o -> o p")[None])
                nc.sync.dma_start(
                    out=gf_row[0:1, 1024:2048].rearrange("a (o p) -> a o p",
                                                         p=128),
                    in_=dxs[:].rearrange("p o -> o p")[None])

            # ws1 (rank-1 gf term via K=1 matmul) -> ws2 -> ws3 ->
            # normalize -> store, fully streamed per 512-chunk
            with tc.tile_pool(name="sps", bufs=2, space="PSUM") as sps:
                for mc in range(MC):
                    sl = slice(mc * CH, (mc + 1) * CH)
                    xs1c = st.tile([128, 4, CH], F32, tag="xs1c", bufs=1,
                                   name=f"xs1c{mc}")
                    for ot in range(4):
                        ps = sps.tile([128, CH], F32, tag="s_ps",
                                      name=f"s1_{ot}_{mc}")
                        for kt in range(8):
                            nc.tensor.matmul(
                                ps,
                                lhsT=r32(ws1k[kt][:, ot * 128:(ot + 1) * 128]),
                                rhs=r32(xf[:, kt, sl]),
                                start=(kt == 0), stop=False)
                        nc.tensor.matmul(
                            ps, lhsT=r32(s1row[0:1, ot * 128:(ot + 1) * 128]),
                            rhs=r32(gf_row[0:1, sl]), start=False, stop=True)
                        nc.scalar.activation(
                            rw(xs1c[:, ot, :]), ps, AF.Relu,
                            bias=small["bns1c"][:, ot:ot + 1],
                            scale=small["bns1s"][:, ot:ot + 1])
                    xs2c = st.tile([128, 2, CH], F32, tag="xs2c", bufs=1,
                                   name=f"xs2c{mc}")
                    for ot in range(2):
                        ps = sps.tile([128, CH], F32, tag="s_ps",
                                      name=f"s2_{ot}_{mc}")
                        for kt in range(4):
                            nc.tensor.matmul(
                                ps,
                                lhsT=r32(ws2T[:, kt, ot * 128:(ot + 1) * 128]),
                                rhs=r32(xs1c[:, kt, :]),
                                start=(kt == 0), stop=(kt == 3))
                        nc.scalar.activation(
                            rw(xs2c[:, ot, :]), ps, AF.Relu,
                            bias=small["bns2c"][:, ot:ot + 1],
                            scale=small["bns2s"][:, ot:ot + 1])
                    ps3 = sps.tile([4, CH], F32, tag="s3_ps", bufs=1,
                                   name=f"s3_{mc}")
                    for kt in range(2):
                        nc.tensor.matmul(ps3[0:3, :], lhsT=r32(ws3T[:, kt, :]),
                                         rhs=r32(xs2c[:, kt, :]),
                                         start=(kt == 0), stop=False)
                    nc.tensor.matmul(ps3[0:3, :], lhsT=r32(small["bs3row"]),
                                     rhs=r32(onesrow[0:1, :]),
                                     start=False, stop=True)
                    o3c = st.tile([3, CH], F32, tag="o3c", bufs=2,
                                  name=f"o3c{mc}")
                    nc.vector.tensor_copy(o3c, ps3[0:3, :])
                    # L2 normalize over the 3 channels and store
                    sq = st.tile([3, CH], F32, tag="sq", bufs=1,
                                 name=f"sq{mc}")
                    nc.vector.tensor_mul(sq, o3c, o3c)
                    psn = sps.tile([1, CH], F32, tag="n_ps", bufs=1,
                                   name=f"n_{mc}")
                    nc.tensor.matmul(psn, lhsT=ones3, rhs=sq,
                                     start=True, stop=True)
                    nrm = st.tile([1, CH], F32, tag="nrm", bufs=1,
                                  name=f"nrm{mc}")
                    nc.scalar.activation(nrm, psn, AF.Sqrt)
                    nc.vector.tensor_scalar_max(nrm, nrm, 1e-12)
                    rn = st.tile([1, CH], F32, tag="rn", bufs=1,
                                 name=f"rn{mc}")
                    nc.vector.reciprocal(rn, nrm)
                    psb = sps.tile([3, CH], F32, tag="b_ps", bufs=1,
                                   name=f"b_{mc}")
                    nc.tensor.matmul(psb, lhsT=ones1f[0:1, 0:3], rhs=rn,
                                     start=True, stop=True)
                    of = st.tile([3, CH], F32, tag="of", bufs=2,
                                 name=f"of{mc}")
                    nc.vector.tensor_mul(of, o3c, psb)
                    nc.sync.dma_start(
                        out=dr["yout"][mc * CH:(mc + 1) * CH, :].rearrange(
                            "n c -> c n"),
                        in_=of)


def build(repeat=1):
    nc = bacc.Bacc("TRN2", target_bir_lowering=False)
    dr = {}
    dr["xin"] = nc.dram_tensor("xin", [N, 3], DT_W, kind="ExternalInput")
    dr["yout"] = nc.dram_tensor("yout", [N, 3], F32, kind="ExternalOutput")
    for nm, shp in WEIGHT_SHAPES.items():
        dr[nm] = nc.dram_tensor(nm, list(shp), dram_dtype(nm),
                                kind="ExternalInput")
    with tile.TileContext(nc, pool_alloc_mode="stack") as tc:
        for _ in range(repeat):
            build_program(nc, tc, dr)
    nc.compile()
    return nc


TRACE = False
LAST_RESULT = None


def kernel(x, params):
    global LAST_RESULT
    from concourse.bass_utils import run_bass_kernel_spmd

    x = np.asarray(x, np.float32)
    H = prep_weights(params)
    nc = build()
    in_maps = []
    for i in range(NCORES):
        m = {"xin": round_f32r(np.ascontiguousarray(x[i]))}
        m.update(H)
        in_maps.append(m)
    res = run_bass_kernel_spmd(nc, in_maps, core_ids=list(range(NCORES)),
                               trace=TRACE)
    LAST_RESULT = res
    out = np.stack([res.results[i]["yout"] for i in range(NCORES)], axis=0)
    return out.astype(np.float32)


if __name__ == "__main__":
    nc = build()
    print("built OK")
